# revision 28
# baseline (speedup 1.0000x reference)
"""CharacterAwareAttention TRN2 kernel: LN(q) -> MHA(x, k, v, +mask) -> out-proj.

Sharding: 8 cores = (batch b in {0,1}) x (head-group g in {0..3}, 4 heads each).
Each core computes, for its (b, 4 heads): LayerNorm(queries), q/k/v projections,
and masked attention, emitting per-head unnormalized context + softmax
denominator [65, Q]. Host divides by the denominator, applies the output
projection per head-group, sums the partials, and adds residual + biases.

Device layout notes:
  - Scores are computed transposed (S^T [k, q]) so the softmax denominator
    falls out of the PV matmul via a ones-augmented V column.
  - Heads are processed in pairs (t in {0,1}); the two heads' score matmuls
    contract over disjoint 64-partition row groups with explicit
    tile_position, so the PE runs them concurrently (row tiling). Both land
    in one [128, 1024] PSUM tile (head_even | head_odd for one q-half)
    consumed by a single exp.
  - k/v projections run in fp8(e4m3) DoubleRow mode: raw keys/values and
    wk/wv are cast to fp8 on the host (weights pre-scaled x16; the inverse
    is folded into wq for kh, and into the host normalization for vh), so
    each matmul contracts 256 rows. fp8 rounding noise is zero-mean and
    independent across the 4096-key softmax average, so it cancels.
  - keys/values/weights are stored in HBM pre-arranged so every partition
    reads one contiguous block (8KB descriptors, full DMA bandwidth).
  - The PE queue is software-pipelined: scores for iteration k+1 are emitted
    before the PV of iteration k, so the PE never head-of-line blocks on the
    exp chain and the HAM clock gate stays at full rate.
  - mask is applied multiplicatively after exp: exp(S + m) = exp(S) * f,
    f in {1, e}, precomputed on host as bf16 [K, Q]; one SBUF copy per
    k-block shared by all 4 heads.
"""

import numpy as np
import ml_dtypes

import concourse.bass as bass
import concourse.tile as tile
from concourse import bacc, mybir
from concourse.bass_utils import run_bass_kernel_spmd
from concourse.masks import make_identity

F32 = mybir.dt.float32
BF16 = mybir.dt.bfloat16
FP8 = mybir.dt.float8e4
AF = mybir.ActivationFunctionType
ALU = mybir.AluOpType
AX = mybir.AxisListType
DR = mybir.MatmulPerfMode.DoubleRow

B, Q, K, D, H = 2, 1024, 4096, 1024, 16
DH = D // H          # 64
NH = 4               # heads per core
HD = NH * DH         # 256, per-core head width
LN_EPS = 1e-5
P = 128
N_CORES = 8
W_SCALE = 16.0       # host pre-scale on wk/wv before fp8 cast

_cached = {}


def _build_program(bands, has_bias):
    """bands: tuple of 32 (qlo, qhi) pairs (empty = (0, 0)) marking, per
    128-row k-block, the q-range where the mask factor is not all-ones."""
    key = ("nc", bands, has_bias)
    if key in _cached:
        return _cached[key]

    nc = bacc.Bacc("TRN2", target_bir_lowering=False, debug=False)

    q_in = nc.dram_tensor("queriesb", [Q, D], BF16, kind="ExternalInput").ap()
    k_in = nc.dram_tensor("keysp", [4, P, 8, K // 4], FP8,
                          kind="ExternalInput").ap()
    v_in = nc.dram_tensor("valuesp", [4, P, 8, K // 4], FP8,
                          kind="ExternalInput").ap()
    mft = nc.dram_tensor("maskft", [K, Q], BF16, kind="ExternalInput").ap()
    wqt = nc.dram_tensor("wqt", [P, 8, HD], BF16, kind="ExternalInput").ap()
    wkt = nc.dram_tensor("wkt", [P, 8, HD], FP8, kind="ExternalInput").ap()
    wvt = nc.dram_tensor("wvt", [P, 8, HD], FP8, kind="ExternalInput").ap()
    bq = nc.dram_tensor("bq", [HD, 1], F32, kind="ExternalInput").ap()
    bk = nc.dram_tensor("bk", [HD, 1], F32, kind="ExternalInput").ap()
    ctxout = nc.dram_tensor("ctxout", [NH, 65, Q], BF16,
                            kind="ExternalOutput").ap()

    q_r = q_in.rearrange("(qb p) d -> qb p d", p=P)        # [8, 128, 1024]
    mft_r = mft.rearrange("(kb p) q -> kb p q", p=P)       # [32, 128, 1024]

    from contextlib import ExitStack

    with ExitStack() as ctx:
        tc = ctx.enter_context(tile.TileContext(nc))
        consts = ctx.enter_context(tc.tile_pool(name="consts", bufs=1))
        wpool = ctx.enter_context(tc.tile_pool(name="weights", bufs=1))
        persist = ctx.enter_context(tc.tile_pool(name="persist", bufs=1))
        bigth = ctx.enter_context(tc.tile_pool(name="bigth", bufs=8))
        q_pool = ctx.enter_context(tc.tile_pool(name="qld", bufs=4))
        x_pool = ctx.enter_context(tc.tile_pool(name="xld", bufs=8))
        sq_pool = ctx.enter_context(tc.tile_pool(name="sq", bufs=2))
        stats = ctx.enter_context(tc.tile_pool(name="stats", bufs=2))
        mf_pool = ctx.enter_context(tc.tile_pool(name="mf", bufs=1))
        u_pool = ctx.enter_context(tc.tile_pool(name="uexp", bufs=8))
        ctxu_pool = ctx.enter_context(tc.tile_pool(name="ctxu", bufs=2))
        psX = ctx.enter_context(tc.tile_pool(name="psX", bufs=2, space="PSUM"))
        psC = ctx.enter_context(tc.tile_pool(name="psC", bufs=2, space="PSUM"))
        if True:
            ident = consts.tile([P, P], BF16)
            make_identity(nc, ident[:])
            eps_sb = consts.tile([P, 1], F32, tag="eps")
            nc.gpsimd.memset(eps_sb[:], LN_EPS)

            # queries first on the sync DMA ring (ahead of keys/values):
            # LayerNorm is on the critical path to the q-projection
            q_tiles = []
            for qb in range(8):
                qt = q_pool.tile([P, D], BF16, tag="qld", name=f"qt{qb}")
                nc.sync.dma_start(qt[:], q_r[qb])
                q_tiles.append(qt)

            bq_sb = consts.tile([P, 2], F32, tag="bq")
            bk_sb = consts.tile([P, 2], F32, tag="bk")
            nc.gpsimd.dma_start(bq_sb[:], bq.rearrange("(t p) o -> p (t o)", p=P))
            nc.gpsimd.dma_start(bk_sb[:], bk.rearrange("(t p) o -> p (t o)", p=P))

            wqt_sb = wpool.tile([P, 8, HD], BF16, tag="wqt")
            wkt_sb = wpool.tile([P, 8, HD], FP8, tag="wkt")
            wvt_sb = wpool.tile([P, 8, HD], FP8, tag="wvt")
            nc.gpsimd.dma_start(wvt_sb[:], wvt)
            nc.gpsimd.dma_start(wkt_sb[:], wkt)
            nc.gpsimd.dma_start(wqt_sb[:], wqt)

            mf_tiles = {}
            for kb in range(32):
                qlo, qhi = bands[kb]
                if qhi > qlo:
                    mf_tiles[kb] = mf_pool.tile([P, qhi - qlo], BF16,
                                                tag=f"mf{kb}", name=f"mf{kb}")

            def load_quarter(kq):
                kT = bigth.tile([P, 8, K // 4], FP8, tag="bigth",
                                name=f"kT{kq}")
                nc.sync.dma_start(kT[:], k_in[kq])
                vT = bigth.tile([P, 8, K // 4], FP8, tag="bigth",
                                name=f"vT{kq}")
                nc.sync.dma_start(vT[:], v_in[kq])
                return vT, kT

            # PE warm-up: back-to-back transposes release the HAM clock gate
            warm_ps = psX.tile([P, 512], BF16, tag="psX", name="warm_ps")
            for _ in range(12):
                nc.tensor.transpose(warm_ps[:, 0:P], ident[:], ident[:])

            xT_sb = persist.tile([P, 8, Q], BF16, tag="xT")
            qhT_sb = persist.tile([P, 2, Q], BF16, tag="qhT")
            khT_sb = persist.tile([P, 2, K], BF16, tag="khT")
            # per-head 66-wide slots: cols h*66..+63 = vh, col h*66+64 = ones
            # (denominator); col 65 is inter-head padding, never read.
            vh_sb = persist.tile([P, 32, NH, 66], BF16, tag="vh")
            vh_r = vh_sb[:]
            # write the ones columns, one quarter at a time so quarter 0 is
            # ready before the first PV matmul
            for kq in range(4):
                ksl = slice(kq * 8, (kq + 1) * 8)
                nc.gpsimd.memset(vh_r[:, ksl, :, 64:65], 1.0)

            # ---- LayerNorm stats + normalized x (bf16) ----
            x_tiles = []

            def emit_ln_stats(qb):
                qt = q_tiles[qb]
                s1 = stats.tile([P, 1], F32, tag="s1")
                nc.vector.reduce_sum(s1[:], qt[:], axis=AX.X)
                sq = sq_pool.tile([P, D], BF16)
                s2 = stats.tile([P, 1], F32, tag="s2")
                nc.scalar.activation(sq[:], qt[:], AF.Square, accum_out=s2[:])
                mu = stats.tile([P, 1], F32, tag="mu")
                nc.vector.tensor_scalar_mul(mu[:], s1[:], 1.0 / D)
                ex2 = stats.tile([P, 1], F32, tag="ex2")
                nc.vector.tensor_scalar_mul(ex2[:], s2[:], 1.0 / D)
                mu2 = stats.tile([P, 1], F32, tag="mu2")
                nc.vector.tensor_mul(mu2[:], mu[:], mu[:])
                var = stats.tile([P, 1], F32, tag="var")
                nc.vector.tensor_sub(var[:], ex2[:], mu2[:])
                std = stats.tile([P, 1], F32, tag="std")
                nc.scalar.activation(std[:], var[:], AF.Sqrt, bias=eps_sb[:])
                rstd = stats.tile([P, 1], F32, tag="rstd")
                nc.vector.reciprocal(rstd[:], std[:])
                nmr = stats.tile([P, 1], F32, tag="nmr")
                nc.vector.tensor_mul(nmr[:], mu[:], rstd[:])
                nc.vector.tensor_scalar_mul(nmr[:], nmr[:], -1.0)
                x = x_pool.tile([P, D], BF16, tag="x", name=f"x{qb}")
                nc.vector.tensor_scalar(x[:], qt[:], rstd[:], nmr[:],
                                        op0=ALU.mult, op1=ALU.add)
                x_tiles.append(x)

            def emit_xt(qb):
                x = x_tiles[qb]
                for g in range(2):
                    px = psX.tile([P, 512], BF16, tag="psX", name="px")
                    for di in range(4):
                        d = g * 4 + di
                        nc.tensor.transpose(px[:, di * P:(di + 1) * P],
                                            x[:, d * P:(d + 1) * P], ident[:])
                    nc.vector.tensor_copy(
                        xT_sb[:, g * 4:(g + 1) * 4, qb * P:(qb + 1) * P],
                        px[:].rearrange("p (a b) -> p a b", b=P),
                    )

            def emit_qht(t, half):
                qsl = slice(half * 512, (half + 1) * 512)
                psq = psX.tile([P, Q], F32, tag="psX", name="psq")
                for dc in range(8):
                    nc.tensor.matmul(
                        psq[:, 0:512],
                        wqt_sb[:, dc, t * P:(t + 1) * P],
                        xT_sb[:, dc, qsl],
                        start=(dc == 0), stop=(dc == 7),
                    )
                nc.vector.tensor_scalar_add(qhT_sb[:, t, qsl], psq[:, 0:512],
                                            bq_sb[:, t:t + 1])

            # ---- fp8 DoubleRow k/v projections (contract 256 rows/matmul) --
            def emit_vproj_group(vT, kq, kb8, pair):
                """vh for one head pair (128 columns of wv) of one k-block."""
                kb = kq * 8 + kb8
                ps_vh = psX.tile([P, Q], F32, tag="psX", name="ps_vh")
                for dc in range(8):
                    nc.tensor.matmul(
                        ps_vh[:, 0:P],
                        vT[:, dc, kb8 * P:(kb8 + 1) * P],
                        wvt_sb[:, dc, pair * P:(pair + 1) * P],
                        start=(dc == 0), stop=(dc == 7),
                    )
                nc.vector.tensor_copy(
                    vh_r[:, kb, 2 * pair:2 * pair + 2, 0:64],
                    ps_vh[:, 0:P].rearrange("p (h c) -> p h c", c=64),
                )

            def emit_kproj_group(kT, kq, t, kc):
                psk = psX.tile([P, Q], F32, tag="psX", name="psk")
                for dcp in range(4):
                    nc.tensor.matmul(
                        psk[:, 0:512],
                        wkt_sb[:, 2 * dcp:2 * dcp + 2, t * P:(t + 1) * P],
                        kT[:, 2 * dcp:2 * dcp + 2, kc * 512:(kc + 1) * 512],
                        start=(dcp == 0), stop=(dcp == 3),
                        perf_mode=DR,
                    )
                ksl = slice(kq * 1024 + kc * 512, kq * 1024 + (kc + 1) * 512)
                nc.vector.tensor_scalar_add(khT_sb[:, t, ksl], psk[:, 0:512],
                                            bk_sb[:, t:t + 1])

            # ---- attention: paired scores -> exp -> mask -> PV ----
            ctx_ps = {}

            def emit_scores_pair(t, kb, half):
                """Row-tiled concurrent scores for heads (2t, 2t+1) on one
                q-half; both land in one PSUM tile -> one exp -> u."""
                qsl = slice(half * 512, (half + 1) * 512)
                ps_s = psX.tile([P, Q], F32, tag="psX", name="ps_s")
                for r, col in ((0, 0), (64, 512)):
                    nc.tensor.matmul(
                        ps_s[:, col:col + 512],
                        khT_sb[r:r + 64, t, kb * P:(kb + 1) * P],
                        qhT_sb[r:r + 64, t, qsl],
                        start=True, stop=True,
                        tile_position=(r, 0),
                    )
                u = u_pool.tile([P, Q], BF16, name="u")
                nc.scalar.activation(u[:], ps_s[:], AF.Exp)
                qlo, qhi = bands[kb]
                a = max(qlo, half * 512)
                b_ = min(qhi, (half + 1) * 512)
                if b_ > a:
                    mf = mf_tiles[kb]
                    for col in (0, 512):
                        usl = slice(col + a - half * 512, col + b_ - half * 512)
                        nc.vector.tensor_mul(u[:, usl], u[:, usl],
                                             mf[:, a - qlo:b_ - qlo])
                return u

            def emit_pv_pair(t, kb, half, u):
                qsl = slice(half * 512, (half + 1) * 512)
                for hh, col in ((0, 0), (1, 512)):
                    h = 2 * t + hh
                    nc.tensor.matmul(
                        ctx_ps[h][0:65, qsl],
                        vh_r[:, kb, h, 0:65],
                        u[:, col:col + 512],
                        start=(kb == 0), stop=(kb == 31),
                    )

            def drain_head(h):
                ctxut = ctxu_pool.tile([P, Q], BF16, name="ctxut")
                nc.vector.tensor_copy(ctxut[0:65, :], ctx_ps[h][0:65, :])
                nc.gpsimd.dma_start(ctxout[h], ctxut[0:65, :])

            # ================== emission schedule ==================
            vT0, kT0 = load_quarter(0)
            for qb in range(8):
                emit_ln_stats(qb)
            # earliest possible q-projection: half 0 after xT of qb 0-3
            for qb in range(4):
                emit_xt(qb)
            for kc in range(2):
                emit_kproj_group(kT0, 0, 0, kc)
            emit_qht(0, 0)
            for qb in range(4, 8):
                emit_xt(qb)
            emit_qht(0, 1)
            for kb8 in range(8):
                emit_vproj_group(vT0, 0, kb8, 0)
            vT1, kT1 = load_quarter(1)
            for kb in list(mf_tiles):
                qlo, qhi = bands[kb]
                nc.gpsimd.dma_start(mf_tiles[kb][:], mft_r[kb][:, qlo:qhi])

            quarters = {0: (vT0, kT0), 1: (vT1, kT1)}
            proj_work = []    # must all complete within phase A

            def queue_quarter_proj(kq):
                vT, kT = quarters[kq]
                for kb8 in range(8):
                    proj_work.append(
                        lambda vT=vT, kq=kq, kb8=kb8:
                        emit_vproj_group(vT, kq, kb8, 0))
                for kc in range(2):
                    proj_work.append(
                        lambda kT=kT, kq=kq, kc=kc:
                        emit_kproj_group(kT, kq, 0, kc))

            queue_quarter_proj(1)
            # quarter-0 k-projection for the second head pair + its q-proj
            # (qhT t=1 is needed at the very start of phase B)
            proj_work.append(lambda: emit_kproj_group(kT0, 0, 1, 0))
            proj_work.append(lambda: emit_kproj_group(kT0, 0, 1, 1))
            proj_work.append(lambda: emit_qht(1, 0))
            proj_work.append(lambda: emit_qht(1, 1))

            # work deferred into phase B's PE slack, paced by iteration:
            # vh for head pair 1 (one k-block per iteration) and khT t=1
            # for quarters 1-3 (ahead of their consumption)
            defer_b = {kb: [] for kb in range(32)}
            for kb in range(32):
                kq, kb8 = kb // 8, kb % 8
                defer_b[kb].append(
                    lambda kq=kq, kb8=kb8:
                    emit_vproj_group(quarters[kq][0], kq, kb8, 1))
            for kq in range(1, 4):
                for kc in range(2):
                    defer_b[8 * (kq - 1) + 1 + 3 * kc].append(
                        lambda kq=kq, kc=kc:
                        emit_kproj_group(quarters[kq][1], kq, 1, kc))

            # phase A: head pair 0 (heads 0, 1), interleaved with projections
            ctx_ps[0] = psC.tile([P, Q], F32, tag="psC", name="ctx_ps_0")
            ctx_ps[1] = psC.tile([P, Q], F32, tag="psC", name="ctx_ps_1")
            u_prev = [emit_scores_pair(0, 0, 0), emit_scores_pair(0, 0, 1)]
            for kb in range(32):
                if kb == 2:
                    quarters[2] = load_quarter(2)
                    queue_quarter_proj(2)
                if kb == 10:
                    quarters[3] = load_quarter(3)
                    queue_quarter_proj(3)
                nwork = 2 if kb < 26 else 0
                for _ in range(nwork):
                    if proj_work:
                        proj_work.pop(0)()
                u_next = None
                if kb < 31:
                    u_next = [emit_scores_pair(0, kb + 1, 0),
                              emit_scores_pair(0, kb + 1, 1)]
                for half in range(2):
                    emit_pv_pair(0, kb, half, u_prev[half])
                u_prev = u_next
            while proj_work:
                proj_work.pop(0)()
            drain_head(0)
            drain_head(1)

            # phase B: head pair 1 (heads 2, 3); deferred khT t=1 projections
            # for quarters 1-3 ride in the PE slack under the exp wall
            ctx_ps[2] = psC.tile([P, Q], F32, tag="psC", name="ctx_ps_2")
            ctx_ps[3] = psC.tile([P, Q], F32, tag="psC", name="ctx_ps_3")
            u_prev = [emit_scores_pair(1, 0, 0), emit_scores_pair(1, 0, 1)]
            for kb in range(32):
                for work in defer_b[kb]:
                    work()
                u_next = None
                if kb < 31:
                    u_next = [emit_scores_pair(1, kb + 1, 0),
                              emit_scores_pair(1, kb + 1, 1)]
                for half in range(2):
                    emit_pv_pair(1, kb, half, u_prev[half])
                u_prev = u_next
            drain_head(2)
            drain_head(3)

    nc.compile()
    _cached[key] = nc
    return nc


def _mask_row_intervals(word_boundaries, char_boundaries):
    """Per-query-row mask intervals, mirroring reference.char_aware_mask.
    Returns (valid, [(lo, hi), ...] x3) arrays of shape [Q]."""
    wb = np.asarray(word_boundaries, dtype=np.int64)
    cb = np.asarray(char_boundaries, dtype=np.int64)
    ws, we = wb[:-1], wb[1:]
    nW = ws.shape[0]
    cs = cb[np.clip(ws, 0, Q - 1)]
    ce = cb[np.clip(we - 1, 0, Q - 1)]
    q = np.arange(Q)
    i = np.clip(np.searchsorted(wb, q, side="right") - 1, 0, nW - 1)
    valid = (q >= ws[i]) & (q < we[i])
    iv = []
    iv.append((cs[i], ce[i]))
    ps_ = ws[np.maximum(i - 1, 0)]
    lo1 = np.where(i > 0, ps_, 0)
    hi1 = np.where(i > 0, ws[i], 0)
    iv.append((lo1, hi1))
    ns = we[i]
    ne = wb[np.minimum(i + 2, nW)]
    lo2 = np.where(i < nW - 1, ns, 0)
    hi2 = np.where(i < nW - 1, ne, 0)
    iv.append((lo2, hi2))
    return valid, iv


def _mask_factor_T(word_boundaries, char_boundaries):
    """exp(mask)^T [K, Q] as bf16; mirrors reference.char_aware_mask."""
    valid, iv = _mask_row_intervals(word_boundaries, char_boundaries)
    j = np.arange(K)[None, :]
    m = np.zeros((Q, K), bool)
    for lo, hi in iv:
        m |= (j >= lo[:, None]) & (j < hi[:, None])
    mask = valid[:, None] & m
    mf = np.where(mask, np.float32(np.e), np.float32(1.0))
    return np.ascontiguousarray(mf.T).astype(ml_dtypes.bfloat16)


def _mask_bands(word_boundaries, char_boundaries):
    """Per 128-row k-block, the [qlo, qhi) range of query columns whose mask
    row intersects the block (16-aligned); (0, 0) when none do."""
    valid, iv = _mask_row_intervals(word_boundaries, char_boundaries)
    bands = []
    for kb in range(K // P):
        klo, khi = kb * P, (kb + 1) * P
        touched = np.zeros(Q, bool)
        for lo, hi in iv:
            touched |= (lo < khi) & (hi > klo) & (lo < hi)
        touched &= valid
        idx = np.nonzero(touched)[0]
        if len(idx) == 0:
            bands.append((0, 0))
        else:
            qlo = int(idx[0]) // 16 * 16
            qhi = min(Q, -(-(int(idx[-1]) + 1) // 16) * 16)
            bands.append((qlo, qhi))
    return tuple(bands)


def _contig_pdc(w):
    """[D, N] -> [P, 8, N] so each partition's data is contiguous."""
    return np.ascontiguousarray(w.reshape(8, P, -1).transpose(1, 0, 2))


def _prepare_in_maps(queries, keys, values, word_boundaries, char_boundaries,
                     ln_gamma, ln_beta, in_proj_w, in_proj_b, out_w, out_b):
    scale = 1.0 / np.sqrt(np.float32(DH))
    wq, wk, wv = (in_proj_w[0:D], in_proj_w[D:2 * D], in_proj_w[2 * D:3 * D])
    bq_full, bk_full, bv_full = (in_proj_b[0:D], in_proj_b[D:2 * D],
                                 in_proj_b[2 * D:3 * D])

    maskft = _mask_factor_T(word_boundaries, char_boundaries)
    # kv pre-arranged [4, P, 8, K/4] fp8: quarter kq, partition p reads rows
    # dc*128+p of keys^T [D, K]; contiguous 8KB per partition per quarter.
    fp8 = ml_dtypes.float8_e4m3

    def kv_pre(a):  # a: [B, K, D] f32
        at = a.astype(fp8).transpose(0, 2, 1)          # [B, D, K]
        at = at.reshape(B, 8, P, 4, K // 4)             # dc, p, kq, k
        return np.ascontiguousarray(at.transpose(0, 3, 2, 1, 4))  # [B,4,P,8,k]

    keys_p = kv_pre(keys)
    values_p = kv_pre(values)
    queries_b = queries.astype(ml_dtypes.bfloat16)

    in_maps = []
    for c in range(N_CORES):
        b, g = c // 4, c % 4
        hsl = slice(g * HD, (g + 1) * HD)
        wq_g = wq[hsl].astype(np.float32)
        # fold LN gamma, the attention scale, and 1/W_SCALE (compensating the
        # fp8 pre-scale on wk) into wq; fold beta into the q bias
        wqt_g = (wq_g * ln_gamma[None, :] * (scale / W_SCALE)).T
        bq_g = (scale / W_SCALE) * (wq_g @ ln_beta + bq_full[hsl])
        wk_g = (wk[hsl].astype(np.float32) * W_SCALE).T     # [D, HD]
        wv_g = (wv[hsl].astype(np.float32) * W_SCALE).T
        in_maps.append({
            "queriesb": queries_b[b],
            "keysp": keys_p[b],
            "valuesp": values_p[b],
            "maskft": maskft,
            "wqt": _contig_pdc(wqt_g.astype(ml_dtypes.bfloat16)),
            "wkt": _contig_pdc(wk_g.astype(fp8)),
            "wvt": _contig_pdc(wv_g.astype(fp8)),
            "bq": bq_g.reshape(HD, 1).astype(np.float32),
            "bk": (bk_full[hsl] * W_SCALE).reshape(HD, 1).astype(np.float32),
        })
    return in_maps


def _install_trace_shims():
    """Make run_bass_kernel_spmd(trace=True) work in this container: provide
    the missing antenv.axon_hooks module (backed by the axon .so's NRT
    profile C-ABI) and skip the S3 artifact upload."""
    import sys, types
    if "antenv.axon_hooks" not in sys.modules:
        from trn_agent_boot.trn_boot import _ntff_profile_via_ctypes
        hook = _ntff_profile_via_ctypes("/opt/axon/libaxon_pjrt.so")
        mod = types.ModuleType("antenv.axon_hooks")
        mod.get_axon_ntff_profile_hook = lambda: hook
        sys.modules["antenv.axon_hooks"] = mod
    import concourse.bass_utils as bu
    bu.upload_artifacts = lambda tmpdir: f"local://{tmpdir}"


def run(inputs: dict, trace: bool = False):
    inputs = {k: np.asarray(v) for k, v in inputs.items()}
    if trace:
        _install_trace_shims()
    bands = _mask_bands(inputs["word_boundaries"], inputs["char_boundaries"])
    nc = _build_program(bands, False)
    in_maps = _prepare_in_maps(**inputs)
    res = run_bass_kernel_spmd(nc, in_maps, core_ids=list(range(N_CORES)),
                               trace=trace)
    queries = inputs["queries"].astype(np.float32)
    out_w = inputs["out_w"].astype(np.float32)
    out_b = inputs["out_b"].astype(np.float32)
    # value bias: softmax weights sum to 1, so bv adds exactly bv@outw^T
    bv_full = inputs["in_proj_b"][2 * D:3 * D].astype(np.float32)
    bv_term = bv_full @ out_w.T
    full = np.empty((B, Q, D), np.float32)
    for b in range(B):
        acc = queries[b] + (out_b + bv_term)[None, :]
        for g in range(4):
            ctx = res.results[4 * b + g]["ctxout"].astype(np.float32)
            # ctx[h]: rows 0:64 = unnormalized head ctx^T (x W_SCALE),
            # row 64 = denom
            p = ctx[:, 0:64, :] / (ctx[:, 64:65, :] * W_SCALE)  # [4, 64, Q]
            ctxn = p.transpose(2, 0, 1).reshape(Q, HD)           # [Q, 256]
            acc = acc + ctxn @ out_w[:, g * HD:(g + 1) * HD].T
        full[b] = acc
    return full, res


def kernel(**inputs) -> np.ndarray:
    out, _ = run(inputs)
    return out


# revision 30
# speedup vs baseline: 1.0307x; 1.0307x over previous
"""CharacterAwareAttention TRN2 kernel: LN(q) -> MHA(x, k, v, +mask) -> out-proj.

Sharding: 8 cores = (batch b in {0,1}) x (head-group g in {0..3}, 4 heads each).
Each core computes, for its (b, 4 heads): LayerNorm(queries), q/k/v projections,
and masked attention, emitting per-head unnormalized context + softmax
denominator [65, Q]. Host divides by the denominator, applies the output
projection per head-group, sums the partials, and adds residual + biases.

Device layout notes:
  - Scores are computed transposed (S^T [k, q]) so the softmax denominator
    falls out of the PV matmul via a ones-augmented V column.
  - Heads are processed in pairs (t in {0,1}); the two heads' score matmuls
    contract over disjoint 64-partition row groups with explicit
    tile_position, so the PE runs them concurrently (row tiling). Both land
    in one [128, 1024] PSUM tile (head_even | head_odd for one q-half)
    consumed by a single exp.
  - k/v projections run in fp8(e4m3) DoubleRow mode: raw keys/values and
    wk/wv are cast to fp8 on the host (weights pre-scaled x16; the inverse
    is folded into wq for kh, and into the host normalization for vh), so
    each matmul contracts 256 rows. fp8 rounding noise is zero-mean and
    independent across the 4096-key softmax average, so it cancels.
  - keys/values/weights are stored in HBM pre-arranged so every partition
    reads one contiguous block (8KB descriptors, full DMA bandwidth).
  - The PE queue is software-pipelined: scores for iteration k+1 are emitted
    before the PV of iteration k, so the PE never head-of-line blocks on the
    exp chain and the HAM clock gate stays at full rate.
  - mask is applied multiplicatively after exp: exp(S + m) = exp(S) * f,
    f in {1, e}, precomputed on host as bf16 [K, Q]; one SBUF copy per
    k-block shared by all 4 heads.
"""

import numpy as np
import ml_dtypes

import concourse.bass as bass
import concourse.tile as tile
from concourse import bacc, mybir
from concourse.bass_utils import run_bass_kernel_spmd
from concourse.masks import make_identity

F32 = mybir.dt.float32
BF16 = mybir.dt.bfloat16
FP8 = mybir.dt.float8e4
AF = mybir.ActivationFunctionType
ALU = mybir.AluOpType
AX = mybir.AxisListType
DR = mybir.MatmulPerfMode.DoubleRow

B, Q, K, D, H = 2, 1024, 4096, 1024, 16
DH = D // H          # 64
NH = 4               # heads per core
HD = NH * DH         # 256, per-core head width
LN_EPS = 1e-5
P = 128
N_CORES = 8
W_SCALE = 16.0       # host pre-scale on wk/wv before fp8 cast

_cached = {}


def _build_program(bands, has_bias):
    """bands: tuple of 32 (qlo, qhi) pairs (empty = (0, 0)) marking, per
    128-row k-block, the q-range where the mask factor is not all-ones."""
    key = ("nc", bands, has_bias)
    if key in _cached:
        return _cached[key]

    nc = bacc.Bacc("TRN2", target_bir_lowering=False, debug=False)

    q_in = nc.dram_tensor("queriesb", [Q, D], BF16, kind="ExternalInput").ap()
    k_in = nc.dram_tensor("keysp", [4, P, 8, K // 4], FP8,
                          kind="ExternalInput").ap()
    v_in = nc.dram_tensor("valuesp", [4, P, 8, K // 4], FP8,
                          kind="ExternalInput").ap()
    mft = nc.dram_tensor("maskft", [K, Q], BF16, kind="ExternalInput").ap()
    wqt = nc.dram_tensor("wqt", [P, 8, HD], BF16, kind="ExternalInput").ap()
    wkt = nc.dram_tensor("wkt", [P, 8, HD], FP8, kind="ExternalInput").ap()
    wvt = nc.dram_tensor("wvt", [P, 8, HD], FP8, kind="ExternalInput").ap()
    bq = nc.dram_tensor("bq", [HD, 1], F32, kind="ExternalInput").ap()
    bk = nc.dram_tensor("bk", [HD, 1], F32, kind="ExternalInput").ap()
    ctxout = nc.dram_tensor("ctxout", [NH, 65, Q], BF16,
                            kind="ExternalOutput").ap()

    q_r = q_in.rearrange("(qb p) d -> qb p d", p=P)        # [8, 128, 1024]
    mft_r = mft.rearrange("(kb p) q -> kb p q", p=P)       # [32, 128, 1024]

    from contextlib import ExitStack

    with ExitStack() as ctx:
        tc = ctx.enter_context(tile.TileContext(nc))
        consts = ctx.enter_context(tc.tile_pool(name="consts", bufs=1))
        wpool = ctx.enter_context(tc.tile_pool(name="weights", bufs=1))
        persist = ctx.enter_context(tc.tile_pool(name="persist", bufs=1))
        bigth = ctx.enter_context(tc.tile_pool(name="bigth", bufs=4))
        q_pool = ctx.enter_context(tc.tile_pool(name="qld", bufs=4))
        x_pool = ctx.enter_context(tc.tile_pool(name="xld", bufs=8))
        sq_pool = ctx.enter_context(tc.tile_pool(name="sq", bufs=2))
        stats = ctx.enter_context(tc.tile_pool(name="stats", bufs=2))
        mf_pool = ctx.enter_context(tc.tile_pool(name="mf", bufs=1))
        u_pool = ctx.enter_context(tc.tile_pool(name="uexp", bufs=8))
        ctxu_pool = ctx.enter_context(tc.tile_pool(name="ctxu", bufs=2))
        psX = ctx.enter_context(tc.tile_pool(name="psX", bufs=2, space="PSUM"))
        psC = ctx.enter_context(tc.tile_pool(name="psC", bufs=2, space="PSUM"))
        if True:
            ident = consts.tile([P, P], BF16)
            make_identity(nc, ident[:])
            eps_sb = consts.tile([P, 1], F32, tag="eps")
            nc.gpsimd.memset(eps_sb[:], LN_EPS)

            # queries first on the sync DMA ring (ahead of keys/values):
            # LayerNorm is on the critical path to the q-projection
            q_tiles = []
            for qb in range(8):
                qt = q_pool.tile([P, D], BF16, tag="qld", name=f"qt{qb}")
                nc.sync.dma_start(qt[:], q_r[qb])
                q_tiles.append(qt)

            bq_sb = consts.tile([P, 2], F32, tag="bq")
            bk_sb = consts.tile([P, 2], F32, tag="bk")
            nc.gpsimd.dma_start(bq_sb[:], bq.rearrange("(t p) o -> p (t o)", p=P))
            nc.gpsimd.dma_start(bk_sb[:], bk.rearrange("(t p) o -> p (t o)", p=P))

            wqt_sb = wpool.tile([P, 8, HD], BF16, tag="wqt")
            wkt_sb = wpool.tile([P, 8, HD], FP8, tag="wkt")
            wvt_sb = wpool.tile([P, 8, HD], FP8, tag="wvt")
            nc.gpsimd.dma_start(wvt_sb[:], wvt)
            nc.gpsimd.dma_start(wkt_sb[:], wkt)
            nc.gpsimd.dma_start(wqt_sb[:], wqt)

            mf_tiles = {}
            for kb in range(32):
                qlo, qhi = bands[kb]
                if qhi > qlo:
                    mf_tiles[kb] = mf_pool.tile([P, qhi - qlo], BF16,
                                                tag=f"mf{kb}", name=f"mf{kb}")

            def load_quarter(kq):
                kT = bigth.tile([P, 8, K // 4], FP8, tag="bigth",
                                name=f"kT{kq}")
                nc.sync.dma_start(kT[:], k_in[kq])
                vT = bigth.tile([P, 8, K // 4], FP8, tag="bigth",
                                name=f"vT{kq}")
                nc.sync.dma_start(vT[:], v_in[kq])
                return vT, kT

            # PE warm-up: back-to-back transposes release the HAM clock gate
            warm_ps = psX.tile([P, 512], BF16, tag="psX", name="warm_ps")
            for _ in range(12):
                nc.tensor.transpose(warm_ps[:, 0:P], ident[:], ident[:])

            xT_sb = persist.tile([P, 8, Q], BF16, tag="xT")
            qhT_sb = persist.tile([P, 2, Q], BF16, tag="qhT")
            khT_sb = persist.tile([P, 2, K], BF16, tag="khT")
            # per-head 66-wide slots: cols h*66..+63 = vh, col h*66+64 = ones
            # (denominator); col 65 is inter-head padding, never read.
            vh_sb = persist.tile([P, 32, NH, 66], BF16, tag="vh")
            vh_r = vh_sb[:]
            # write the ones columns, one quarter at a time so quarter 0 is
            # ready before the first PV matmul
            for kq in range(4):
                ksl = slice(kq * 8, (kq + 1) * 8)
                nc.gpsimd.memset(vh_r[:, ksl, :, 64:65], 1.0)

            # ---- LayerNorm stats + normalized x (bf16) ----
            x_tiles = []

            def emit_ln_stats(qb):
                qt = q_tiles[qb]
                s1 = stats.tile([P, 1], F32, tag="s1")
                nc.vector.reduce_sum(s1[:], qt[:], axis=AX.X)
                sq = sq_pool.tile([P, D], BF16)
                s2 = stats.tile([P, 1], F32, tag="s2")
                nc.scalar.activation(sq[:], qt[:], AF.Square, accum_out=s2[:])
                mu = stats.tile([P, 1], F32, tag="mu")
                nc.vector.tensor_scalar_mul(mu[:], s1[:], 1.0 / D)
                ex2 = stats.tile([P, 1], F32, tag="ex2")
                nc.vector.tensor_scalar_mul(ex2[:], s2[:], 1.0 / D)
                mu2 = stats.tile([P, 1], F32, tag="mu2")
                nc.vector.tensor_mul(mu2[:], mu[:], mu[:])
                var = stats.tile([P, 1], F32, tag="var")
                nc.vector.tensor_sub(var[:], ex2[:], mu2[:])
                std = stats.tile([P, 1], F32, tag="std")
                nc.scalar.activation(std[:], var[:], AF.Sqrt, bias=eps_sb[:])
                rstd = stats.tile([P, 1], F32, tag="rstd")
                nc.vector.reciprocal(rstd[:], std[:])
                nmr = stats.tile([P, 1], F32, tag="nmr")
                nc.vector.tensor_mul(nmr[:], mu[:], rstd[:])
                nc.vector.tensor_scalar_mul(nmr[:], nmr[:], -1.0)
                x = x_pool.tile([P, D], BF16, tag="x", name=f"x{qb}")
                nc.vector.tensor_scalar(x[:], qt[:], rstd[:], nmr[:],
                                        op0=ALU.mult, op1=ALU.add)
                x_tiles.append(x)

            def emit_xt(qb):
                x = x_tiles[qb]
                for g in range(2):
                    px = psX.tile([P, 512], BF16, tag="psX", name="px")
                    for di in range(4):
                        d = g * 4 + di
                        nc.tensor.transpose(px[:, di * P:(di + 1) * P],
                                            x[:, d * P:(d + 1) * P], ident[:])
                    nc.vector.tensor_copy(
                        xT_sb[:, g * 4:(g + 1) * 4, qb * P:(qb + 1) * P],
                        px[:].rearrange("p (a b) -> p a b", b=P),
                    )

            def emit_qht(t, half):
                qsl = slice(half * 512, (half + 1) * 512)
                psq = psX.tile([P, Q], F32, tag="psX", name="psq")
                for dc in range(8):
                    nc.tensor.matmul(
                        psq[:, 0:512],
                        wqt_sb[:, dc, t * P:(t + 1) * P],
                        xT_sb[:, dc, qsl],
                        start=(dc == 0), stop=(dc == 7),
                    )
                nc.vector.tensor_scalar_add(qhT_sb[:, t, qsl], psq[:, 0:512],
                                            bq_sb[:, t:t + 1])

            # ---- fp8 DoubleRow k/v projections (contract 256 rows/matmul) --
            def emit_vproj_group(vT, kq, kb8):
                kb = kq * 8 + kb8
                ps_vh = psX.tile([P, Q], F32, tag="psX", name="ps_vh")
                for dc in range(8):
                    nc.tensor.matmul(
                        ps_vh[:, 0:HD],
                        vT[:, dc, kb8 * P:(kb8 + 1) * P],
                        wvt_sb[:, dc, :],
                        start=(dc == 0), stop=(dc == 7),
                    )
                nc.vector.tensor_copy(
                    vh_r[:, kb, :, 0:64],
                    ps_vh[:, 0:HD].rearrange("p (h c) -> p h c", c=64),
                )

            def emit_kproj_group(kT, kq, t, kc):
                psk = psX.tile([P, Q], F32, tag="psX", name="psk")
                for dcp in range(4):
                    nc.tensor.matmul(
                        psk[:, 0:512],
                        wkt_sb[:, 2 * dcp:2 * dcp + 2, t * P:(t + 1) * P],
                        kT[:, 2 * dcp:2 * dcp + 2, kc * 512:(kc + 1) * 512],
                        start=(dcp == 0), stop=(dcp == 3),
                        perf_mode=DR,
                    )
                ksl = slice(kq * 1024 + kc * 512, kq * 1024 + (kc + 1) * 512)
                nc.vector.tensor_scalar_add(khT_sb[:, t, ksl], psk[:, 0:512],
                                            bk_sb[:, t:t + 1])

            # ---- attention: paired scores -> exp -> mask -> PV ----
            ctx_ps = {}

            def emit_scores_pair(t, kb, half):
                """Row-tiled concurrent scores for heads (2t, 2t+1) on one
                q-half; both land in one PSUM tile -> one exp -> u."""
                qsl = slice(half * 512, (half + 1) * 512)
                ps_s = psX.tile([P, Q], F32, tag="psX", name="ps_s")
                for r, col in ((0, 0), (64, 512)):
                    nc.tensor.matmul(
                        ps_s[:, col:col + 512],
                        khT_sb[r:r + 64, t, kb * P:(kb + 1) * P],
                        qhT_sb[r:r + 64, t, qsl],
                        start=True, stop=True,
                        tile_position=(r, 0),
                    )
                u = u_pool.tile([P, Q], BF16, name="u")
                nc.scalar.activation(u[:], ps_s[:], AF.Exp)
                qlo, qhi = bands[kb]
                a = max(qlo, half * 512)
                b_ = min(qhi, (half + 1) * 512)
                if b_ > a:
                    mf = mf_tiles[kb]
                    for col in (0, 512):
                        usl = slice(col + a - half * 512, col + b_ - half * 512)
                        nc.vector.tensor_mul(u[:, usl], u[:, usl],
                                             mf[:, a - qlo:b_ - qlo])
                return u

            def emit_pv_pair(t, kb, half, u):
                qsl = slice(half * 512, (half + 1) * 512)
                for hh, col in ((0, 0), (1, 512)):
                    h = 2 * t + hh
                    nc.tensor.matmul(
                        ctx_ps[h][0:65, qsl],
                        vh_r[:, kb, h, 0:65],
                        u[:, col:col + 512],
                        start=(kb == 0), stop=(kb == 31),
                    )

            def drain_head(h):
                ctxut = ctxu_pool.tile([P, Q], BF16, name="ctxut")
                nc.vector.tensor_copy(ctxut[0:65, :], ctx_ps[h][0:65, :])
                nc.gpsimd.dma_start(ctxout[h], ctxut[0:65, :])

            # ================== emission schedule ==================
            vT0, kT0 = load_quarter(0)
            for qb in range(8):
                emit_ln_stats(qb)
            # earliest possible q-projection: half 0 after xT of qb 0-3
            for qb in range(4):
                emit_xt(qb)
            for kc in range(2):
                emit_kproj_group(kT0, 0, 0, kc)
            emit_qht(0, 0)
            for qb in range(4, 8):
                emit_xt(qb)
            emit_qht(0, 1)
            # only the first k-block's vh before attention; the rest of
            # quarter 0 interleaves into the sweep (deps keep PV correct)
            emit_vproj_group(vT0, 0, 0)
            vT1, kT1 = load_quarter(1)
            for kb in list(mf_tiles):
                qlo, qhi = bands[kb]
                nc.gpsimd.dma_start(mf_tiles[kb][:], mft_r[kb][:, qlo:qhi])

            quarters = {1: (vT1, kT1)}
            proj_work = []    # must all complete within phase A
            for kb8 in range(1, 8):
                proj_work.append(
                    lambda kb8=kb8: emit_vproj_group(vT0, 0, kb8))

            def queue_quarter_proj(kq):
                vT, kT = quarters[kq]
                for kb8 in range(8):
                    proj_work.append(
                        lambda vT=vT, kq=kq, kb8=kb8:
                        emit_vproj_group(vT, kq, kb8))
                for t in range(2):
                    for kc in range(2):
                        proj_work.append(
                            lambda kT=kT, kq=kq, t=t, kc=kc:
                            emit_kproj_group(kT, kq, t, kc))

            queue_quarter_proj(1)
            # quarter-0 k-projection for the second head pair + its q-proj
            # (qhT t=1 is needed at the very start of phase B)
            proj_work.append(lambda: emit_kproj_group(kT0, 0, 1, 0))
            proj_work.append(lambda: emit_kproj_group(kT0, 0, 1, 1))
            proj_work.append(lambda: emit_qht(1, 0))
            proj_work.append(lambda: emit_qht(1, 1))

            # phase A: head pair 0 (heads 0, 1), interleaved with projections
            ctx_ps[0] = psC.tile([P, Q], F32, tag="psC", name="ctx_ps_0")
            ctx_ps[1] = psC.tile([P, Q], F32, tag="psC", name="ctx_ps_1")
            u_prev = [emit_scores_pair(0, 0, 0), emit_scores_pair(0, 0, 1)]
            for kb in range(32):
                if kb == 2:
                    quarters[2] = load_quarter(2)
                    queue_quarter_proj(2)
                if kb == 10:
                    quarters[3] = load_quarter(3)
                    queue_quarter_proj(3)
                nwork = 2 if kb < 26 else 0
                for _ in range(nwork):
                    if proj_work:
                        proj_work.pop(0)()
                u_next = None
                if kb < 31:
                    u_next = [emit_scores_pair(0, kb + 1, 0),
                              emit_scores_pair(0, kb + 1, 1)]
                for half in range(2):
                    emit_pv_pair(0, kb, half, u_prev[half])
                u_prev = u_next
            while proj_work:
                proj_work.pop(0)()
            drain_head(0)
            drain_head(1)

            # phase B: head pair 1 (heads 2, 3); deferred khT t=1 projections
            # for quarters 1-3 ride in the PE slack under the exp wall
            ctx_ps[2] = psC.tile([P, Q], F32, tag="psC", name="ctx_ps_2")
            ctx_ps[3] = psC.tile([P, Q], F32, tag="psC", name="ctx_ps_3")
            u_prev = [emit_scores_pair(1, 0, 0), emit_scores_pair(1, 0, 1)]
            for kb in range(32):
                u_next = None
                if kb < 31:
                    u_next = [emit_scores_pair(1, kb + 1, 0),
                              emit_scores_pair(1, kb + 1, 1)]
                for half in range(2):
                    emit_pv_pair(1, kb, half, u_prev[half])
                u_prev = u_next
            drain_head(2)
            drain_head(3)

    nc.compile()
    _cached[key] = nc
    return nc


def _mask_row_intervals(word_boundaries, char_boundaries):
    """Per-query-row mask intervals, mirroring reference.char_aware_mask.
    Returns (valid, [(lo, hi), ...] x3) arrays of shape [Q]."""
    wb = np.asarray(word_boundaries, dtype=np.int64)
    cb = np.asarray(char_boundaries, dtype=np.int64)
    ws, we = wb[:-1], wb[1:]
    nW = ws.shape[0]
    cs = cb[np.clip(ws, 0, Q - 1)]
    ce = cb[np.clip(we - 1, 0, Q - 1)]
    q = np.arange(Q)
    i = np.clip(np.searchsorted(wb, q, side="right") - 1, 0, nW - 1)
    valid = (q >= ws[i]) & (q < we[i])
    iv = []
    iv.append((cs[i], ce[i]))
    ps_ = ws[np.maximum(i - 1, 0)]
    lo1 = np.where(i > 0, ps_, 0)
    hi1 = np.where(i > 0, ws[i], 0)
    iv.append((lo1, hi1))
    ns = we[i]
    ne = wb[np.minimum(i + 2, nW)]
    lo2 = np.where(i < nW - 1, ns, 0)
    hi2 = np.where(i < nW - 1, ne, 0)
    iv.append((lo2, hi2))
    return valid, iv


def _mask_factor_T(word_boundaries, char_boundaries):
    """exp(mask)^T [K, Q] as bf16; mirrors reference.char_aware_mask."""
    valid, iv = _mask_row_intervals(word_boundaries, char_boundaries)
    j = np.arange(K)[None, :]
    m = np.zeros((Q, K), bool)
    for lo, hi in iv:
        m |= (j >= lo[:, None]) & (j < hi[:, None])
    mask = valid[:, None] & m
    mf = np.where(mask, np.float32(np.e), np.float32(1.0))
    return np.ascontiguousarray(mf.T).astype(ml_dtypes.bfloat16)


def _mask_bands(word_boundaries, char_boundaries):
    """Per 128-row k-block, the [qlo, qhi) range of query columns whose mask
    row intersects the block (16-aligned); (0, 0) when none do."""
    valid, iv = _mask_row_intervals(word_boundaries, char_boundaries)
    bands = []
    for kb in range(K // P):
        klo, khi = kb * P, (kb + 1) * P
        touched = np.zeros(Q, bool)
        for lo, hi in iv:
            touched |= (lo < khi) & (hi > klo) & (lo < hi)
        touched &= valid
        idx = np.nonzero(touched)[0]
        if len(idx) == 0:
            bands.append((0, 0))
        else:
            qlo = int(idx[0]) // 16 * 16
            qhi = min(Q, -(-(int(idx[-1]) + 1) // 16) * 16)
            bands.append((qlo, qhi))
    return tuple(bands)


def _contig_pdc(w):
    """[D, N] -> [P, 8, N] so each partition's data is contiguous."""
    return np.ascontiguousarray(w.reshape(8, P, -1).transpose(1, 0, 2))


def _prepare_in_maps(queries, keys, values, word_boundaries, char_boundaries,
                     ln_gamma, ln_beta, in_proj_w, in_proj_b, out_w, out_b):
    scale = 1.0 / np.sqrt(np.float32(DH))
    wq, wk, wv = (in_proj_w[0:D], in_proj_w[D:2 * D], in_proj_w[2 * D:3 * D])
    bq_full, bk_full, bv_full = (in_proj_b[0:D], in_proj_b[D:2 * D],
                                 in_proj_b[2 * D:3 * D])

    maskft = _mask_factor_T(word_boundaries, char_boundaries)
    # kv pre-arranged [4, P, 8, K/4] fp8: quarter kq, partition p reads rows
    # dc*128+p of keys^T [D, K]; contiguous 8KB per partition per quarter.
    fp8 = ml_dtypes.float8_e4m3

    def kv_pre(a):  # a: [B, K, D] f32
        at = a.astype(fp8).transpose(0, 2, 1)          # [B, D, K]
        at = at.reshape(B, 8, P, 4, K // 4)             # dc, p, kq, k
        return np.ascontiguousarray(at.transpose(0, 3, 2, 1, 4))  # [B,4,P,8,k]

    keys_p = kv_pre(keys)
    values_p = kv_pre(values)
    queries_b = queries.astype(ml_dtypes.bfloat16)

    in_maps = []
    for c in range(N_CORES):
        b, g = c // 4, c % 4
        hsl = slice(g * HD, (g + 1) * HD)
        wq_g = wq[hsl].astype(np.float32)
        # fold LN gamma, the attention scale, and 1/W_SCALE (compensating the
        # fp8 pre-scale on wk) into wq; fold beta into the q bias
        wqt_g = (wq_g * ln_gamma[None, :] * (scale / W_SCALE)).T
        bq_g = (scale / W_SCALE) * (wq_g @ ln_beta + bq_full[hsl])
        wk_g = (wk[hsl].astype(np.float32) * W_SCALE).T     # [D, HD]
        wv_g = (wv[hsl].astype(np.float32) * W_SCALE).T
        in_maps.append({
            "queriesb": queries_b[b],
            "keysp": keys_p[b],
            "valuesp": values_p[b],
            "maskft": maskft,
            "wqt": _contig_pdc(wqt_g.astype(ml_dtypes.bfloat16)),
            "wkt": _contig_pdc(wk_g.astype(fp8)),
            "wvt": _contig_pdc(wv_g.astype(fp8)),
            "bq": bq_g.reshape(HD, 1).astype(np.float32),
            "bk": (bk_full[hsl] * W_SCALE).reshape(HD, 1).astype(np.float32),
        })
    return in_maps


def _install_trace_shims():
    """Make run_bass_kernel_spmd(trace=True) work in this container: provide
    the missing antenv.axon_hooks module (backed by the axon .so's NRT
    profile C-ABI) and skip the S3 artifact upload."""
    import sys, types
    if "antenv.axon_hooks" not in sys.modules:
        from trn_agent_boot.trn_boot import _ntff_profile_via_ctypes
        hook = _ntff_profile_via_ctypes("/opt/axon/libaxon_pjrt.so")
        mod = types.ModuleType("antenv.axon_hooks")
        mod.get_axon_ntff_profile_hook = lambda: hook
        sys.modules["antenv.axon_hooks"] = mod
    import concourse.bass_utils as bu
    bu.upload_artifacts = lambda tmpdir: f"local://{tmpdir}"


def run(inputs: dict, trace: bool = False):
    inputs = {k: np.asarray(v) for k, v in inputs.items()}
    if trace:
        _install_trace_shims()
    bands = _mask_bands(inputs["word_boundaries"], inputs["char_boundaries"])
    nc = _build_program(bands, False)
    in_maps = _prepare_in_maps(**inputs)
    res = run_bass_kernel_spmd(nc, in_maps, core_ids=list(range(N_CORES)),
                               trace=trace)
    queries = inputs["queries"].astype(np.float32)
    out_w = inputs["out_w"].astype(np.float32)
    out_b = inputs["out_b"].astype(np.float32)
    # value bias: softmax weights sum to 1, so bv adds exactly bv@outw^T
    bv_full = inputs["in_proj_b"][2 * D:3 * D].astype(np.float32)
    bv_term = bv_full @ out_w.T
    full = np.empty((B, Q, D), np.float32)
    for b in range(B):
        acc = queries[b] + (out_b + bv_term)[None, :]
        for g in range(4):
            ctx = res.results[4 * b + g]["ctxout"].astype(np.float32)
            # ctx[h]: rows 0:64 = unnormalized head ctx^T (x W_SCALE),
            # row 64 = denom
            p = ctx[:, 0:64, :] / (ctx[:, 64:65, :] * W_SCALE)  # [4, 64, Q]
            ctxn = p.transpose(2, 0, 1).reshape(Q, HD)           # [Q, 256]
            acc = acc + ctxn @ out_w[:, g * HD:(g + 1) * HD].T
        full[b] = acc
    return full, res


def kernel(**inputs) -> np.ndarray:
    out, _ = run(inputs)
    return out
